# revision 1
# baseline (speedup 1.0000x reference)
"""Trainium2 Bass kernel for the bipartite GNN message-passing encoder.

Math (see reference.py):
  A_r = (adj == r), r = 1..5
  An_r = diag(1/sqrt(Nu)) A_r diag(1/sqrt(Nv))   (exact factorization; the
         Csafe guard in the reference only matters where A==0, contributing 0)
  Hu = relu(sum_r An_r @ W_items_r^T)   [NU, M]
  Hv = relu(sum_r An_r^T @ W_users_r^T) [NI, M]
  U  = relu(Hu @ dense_W^T + relu(u_sideFeat @ u_W1^T + u_b1) @ u_W2^T)
  V  = relu(Hv @ dense_W^T + relu(v_sideFeat @ v_W1^T + v_b1) @ v_W2^T)

Sharding: 4 user-groups x 2 item-groups = 8 cores. Core (a, b) holds the
adjacency block adj[a*1000:(a+1)*1000, b*2000:(b+1)*2000] and computes the
partial Hu^T for its 1000 users (partial over items -> AllReduce over the
pair sharing `a`) and the partial Hv^T for its 2000 items (partial over
users -> AllReduce over the quad sharing `b`, split in two pipelined
halves). Degrees (Nu/Nv) are computed on-device with two small
AllReduces; the inner degree scale rides the mask build (dual-op DVE),
the outer degree scale is applied in pass 2. Pass 2 is computed
redundantly inside each reduce group so the SPMD program has no per-core
constants. The msg_W slices are handed to each core pre-transposed
([R, n, M] layout) as part of the host-side sharding.

Engine layout: the MM stream (masks x W^T) is pure back-to-back matmuls
so the PE HAM clock-gate stays at 2.4 GHz; all remaining transposes
(adj^T, side features, small weights) run on the PE during the prefix
window (while the degree AllReduces are in flight) and finish before the
MM stream starts. No DMA-xbar transposes (they hard-hang the device when
concurrent with collectives, and serialize ~1.2us/tile on the issuing
engine). f32->bf16 conversion on ACT; masks on DVE.
"""

import sys

import numpy as np

if "/opt/trn_rl_repo" not in sys.path:
    sys.path.insert(0, "/opt/trn_rl_repo")

import concourse.bacc as bacc  # noqa: E402
import concourse.mybir as mybir  # noqa: E402
import concourse.tile as tile  # noqa: E402
from concourse.masks import make_identity  # noqa: E402

FP = mybir.dt.float32
BF = mybir.dt.bfloat16
I32 = mybir.dt.int32

NU = NI = 4000
R = 5
M = 256
OUT = 75
SIDE = 64
FDIM = 128

GA, GB = 4, 2  # user groups x item groups
BU = NU // GA  # 1000 users per block
BI = NI // GB  # 2000 items per block
NCORES = GA * GB

AF = mybir.ActivationFunctionType
ALU = mybir.AluOpType

PAIR_GROUPS = [[a * GB, a * GB + 1] for a in range(GA)]  # share users (same a)
QUAD_GROUPS = [[b, GB + b, 2 * GB + b, 3 * GB + b] for b in range(GB)]  # same b


def _ptiles(n, p=128):
    return [(s, min(p, n - s)) for s in range(0, n, p)]


UPT = _ptiles(BU)  # 8 tiles over block users
IPT = _ptiles(BI)  # 16 tiles over block items


def build_program():
    from contextlib import ExitStack

    nc = bacc.Bacc("TRN2", target_bir_lowering=False, debug=False, num_devices=NCORES)

    # ---- I/O ----  (wi/wu arrive pre-transposed: [R, n, M])
    adj_blk = nc.dram_tensor("adj_blk", [BU, BI], I32, kind="ExternalInput")
    wi = nc.dram_tensor("wi", [R, BI, M], FP, kind="ExternalInput")
    wu = nc.dram_tensor("wu", [R, BU, M], FP, kind="ExternalInput")
    uf = nc.dram_tensor("uf", [BU, FDIM], FP, kind="ExternalInput")
    vf = nc.dram_tensor("vf", [BI, FDIM], FP, kind="ExternalInput")
    dw = nc.dram_tensor("dw", [OUT, M], FP, kind="ExternalInput")
    uw1 = nc.dram_tensor("uw1", [SIDE, FDIM], FP, kind="ExternalInput")
    ub1 = nc.dram_tensor("ub1", [SIDE, 1], FP, kind="ExternalInput")
    uw2 = nc.dram_tensor("uw2", [OUT, SIDE], FP, kind="ExternalInput")
    vw1 = nc.dram_tensor("vw1", [SIDE, FDIM], FP, kind="ExternalInput")
    vb1 = nc.dram_tensor("vb1", [SIDE, 1], FP, kind="ExternalInput")
    vw2 = nc.dram_tensor("vw2", [OUT, SIDE], FP, kind="ExternalInput")
    u_out = nc.dram_tensor("u_out", [BU, OUT], FP, kind="ExternalOutput")
    v_out = nc.dram_tensor("v_out", [BI, OUT], FP, kind="ExternalOutput")

    with tile.TileContext(nc) as tc, ExitStack() as ctx:
        res = ctx.enter_context(tc.tile_pool(name="res", bufs=1))
        adjp = ctx.enter_context(tc.tile_pool(name="adjp", bufs=1))
        scr = ctx.enter_context(tc.tile_pool(name="scr", bufs=2))
        dram = ctx.enter_context(tc.tile_pool(name="dram", bufs=1, space="DRAM"))
        ps_cs = tc.alloc_tile_pool(name="ps_cs", bufs=4, space="PSUM")
        ps_tr = tc.alloc_tile_pool(name="ps_tr", bufs=2, space="PSUM")

        ones = res.tile([128, 1], BF, tag="ones")
        nc.gpsimd.memset(ones[:], 1.0)
        ident = res.tile([128, 128], BF, tag="ident")
        make_identity(nc, ident[:])

        # =========== Phase 1: adj load/convert, degrees ===========
        adjb = []  # bf16 [128, 2000] resident
        rd_t = []  # row degree [pu, 1] f32 per user ptile
        cs_ps = [
            ps_cs.tile([1, 500], FP, tag="cs", bufs=4, name="cs") for _ in range(4)
        ]
        for t, (s, pu) in enumerate(UPT):
            ab = res.tile([128, 2000], BF, tag=f"adjb{t}", name="ab")
            adjb.append(ab)
            rd = res.tile([128, 1], FP, tag=f"rd{t}", name="rd")
            rd_t.append(rd)
            rdc = []
            for ci, c in enumerate((0, 1000)):
                ai = scr.tile([128, 1000], I32, tag="ai", bufs=5, name="ai")
                nc.sync.dma_start(out=ai[:pu, :], in_=adj_blk[s : s + pu, c : c + 1000])
                nc.scalar.copy(out=ab[:pu, c : c + 1000], in_=ai[:pu, :])
                # nonzero mask (= min(adj,1)) + row-degree partial via accumulate
                nz = scr.tile([128, 1000], BF, tag="nz", bufs=3, name="nz")
                rc = scr.tile([128, 1], FP, tag="rdc", bufs=3, name="rc")
                nc.vector.tensor_scalar(
                    out=nz[:pu, :], in0=ai[:pu, :], scalar1=1.0,
                    scalar2=None, op0=ALU.min,
                )
                nc.vector.tensor_reduce(
                    out=rc[:pu, :], in_=nz[:pu, :], axis=mybir.AxisListType.X,
                    op=ALU.add,
                )
                rdc.append(rc)
                # column-degree partials accumulate in PSUM over user ptiles
                for hi, h in enumerate((0, 500)):
                    nc.tensor.matmul(
                        cs_ps[ci * 2 + hi][:1, :], lhsT=ones[:pu, :1],
                        rhs=nz[:pu, h : h + 500],
                        start=(t == 0), stop=(t == len(UPT) - 1),
                    )
            nc.vector.tensor_tensor(
                out=rd[:pu, :], in0=rdc[0][:pu, :], in1=rdc[1][:pu, :], op=ALU.add
            )

        # degree AllReduces: row (pair) first -- it alone gates the item side
        dram_rd = dram.tile([BU, 1], FP, tag="dram_rd")
        dram_cd = dram.tile([1, BI], FP, tag="dram_cd")
        dram_rd_red = dram.tile([BU, 1], FP, tag="dram_rd_red")
        dram_cd_red = dram.tile([1, BI], FP, tag="dram_cd_red")
        for t, (s, pu) in enumerate(UPT):
            nc.sync.dma_start(out=dram_rd[s : s + pu, :], in_=rd_t[t][:pu, :])
        nc.gpsimd.collective_compute(
            "AllReduce", ALU.add, replica_groups=PAIR_GROUPS,
            ins=[dram_rd.opt()], outs=[dram_rd_red.opt()],
        )
        for q4 in range(4):
            cde = scr.tile([128, 500], FP, tag="ev", bufs=3, name="cde")
            nc.scalar.copy(out=cde[:1, :], in_=cs_ps[q4][:1, :])
            nc.sync.dma_start(
                out=dram_cd[:, q4 * 500 : (q4 + 1) * 500], in_=cde[:1, :]
            )
        nc.gpsimd.collective_compute(
            "AllReduce", ALU.add, replica_groups=QUAD_GROUPS,
            ins=[dram_cd.opt()], outs=[dram_cd_red.opt()],
        )

        def rsqrt_tiles(src_rows, tiles, nm):
            out = []
            for t, (s, p) in enumerate(tiles):
                raw = scr.tile([128, 1], FP, tag="fraw", name="raw")
                nc.sync.dma_start(out=raw[:p, :], in_=src_rows(s, p))
                m1 = scr.tile([128, 1], FP, tag="fm1", name="m1")
                nc.vector.tensor_scalar(
                    out=m1[:p, :], in0=raw[:p, :], scalar1=1.0, scalar2=None,
                    op0=ALU.max,
                )
                sq = scr.tile([128, 1], FP, tag="fsq", name="sq")
                nc.scalar.sqrt(out=sq[:p, :], in_=m1[:p, :])
                fac = res.tile([128, 1], FP, tag=f"{nm}fac{t}", name="fac")
                nc.vector.reciprocal(out=fac[:p, :], in_=sq[:p, :])
                out.append(fac)
            return out

        a_fac = rsqrt_tiles(lambda s, p: dram_rd_red[s : s + p, :], UPT, "a")

        # =========== Phase 3: adj^T via PE transposes (prefix window) ======
        adjT = []  # bf16 [128, 1000] per item ptile
        for t, (s, pi) in enumerate(IPT):
            at = adjp.tile([128, 1000], BF, tag=f"adjT{t}", name="at")
            adjT.append(at)
            pt_ps = ps_tr.tile([128, 1024], BF, tag="trp", name="pt_ps")
            w = 0
            for j, (us, pu) in enumerate(UPT):
                nc.tensor.transpose(
                    pt_ps[:pi, w : w + pu], adjb[j][:pu, s : s + pi], ident[:pu, :pu]
                )
                w += pu
            nc.scalar.copy(out=at[:pi, :], in_=pt_ps[:pi, :BU])

        # =========== Phase 2: W load+convert (pre-transposed on host) ======
        def prep_w(w_dram, tiles, nm):
            outT = [[None for _ in tiles] for _ in range(R)]
            for r in range(R):
                for kt, (s, p) in enumerate(tiles):
                    wf = scr.tile([128, 256], FP, tag="wf", bufs=4, name="wf")
                    nc.scalar.dma_start(out=wf[:p, :], in_=w_dram[r, s : s + p, :])
                    wt = res.tile([128, 256], BF, tag=f"{nm}T{r}_{kt}", name="wt")
                    outT[r][kt] = wt
                    nc.scalar.copy(out=wt[:p, :], in_=wf[:p, :])
            return outT

        wuT = prep_w(wu, UPT, "wu")
        wiT = prep_w(wi, IPT, "wi")

        # release prefix PSUM pools; open MM pool
        ps_tr.release()
        ps_cs.release()
        ps_mm = tc.alloc_tile_pool(name="ps_mm", bufs=4, space="PSUM")

        # DRAM buffers for pass-1 partials
        ICPS = [(0, 1024), (1024, 976)]  # item column splits (ptile-aligned)
        dram_hvT = [
            dram.tile([M, w], FP, tag=f"dram_hvT{i}", name="dhv")
            for i, (c0, w) in enumerate(ICPS)
        ]
        dram_hvT_red = [
            dram.tile([M, w], FP, tag=f"dram_hvT_red{i}", name="dhvr")
            for i, (c0, w) in enumerate(ICPS)
        ]
        dram_huT = dram.tile([M, BU], FP, tag="dram_huT")
        dram_huT_red = dram.tile([M, BU], FP, tag="dram_huT_red")

        # =========== ITEM-side pass 1 ===========
        # HvT[m, i] partial = sum_r sum_u (a_u * mask_r[u,i]) * Wu[r][m,u]
        for icp, (ic0, icw) in enumerate(ICPS):
            chs = [(0, 512), (512, icw - 512)]
            P = [
                [
                    ps_mm.tile([128, 512], FP, tag="p1", bufs=4, name="P")
                    for _ in range(2)
                ]
                for _ in range(2)
            ]
            for r in range(R):
                for kt, (us, pu) in enumerate(UPT):
                    msk = scr.tile([128, 1024], BF, tag="mask", bufs=3, name="msk")
                    nc.vector.tensor_scalar(
                        out=msk[:pu, :icw], in0=adjb[kt][:pu, ic0 : ic0 + icw],
                        scalar1=float(r + 1), scalar2=a_fac[kt][:pu, :],
                        op0=ALU.is_equal, op1=ALU.mult,
                    )
                    first = r == 0 and kt == 0
                    last = r == R - 1 and kt == len(UPT) - 1
                    for mh in range(2):
                        for ic2, (cs0, cw) in enumerate(chs):
                            nc.tensor.matmul(
                                P[ic2][mh][:, :cw],
                                lhsT=wuT[r][kt][:pu, mh * 128 : (mh + 1) * 128],
                                rhs=msk[:pu, cs0 : cs0 + cw],
                                start=first, stop=last,
                            )
            for ic2, (cs0, cw) in enumerate(chs):
                for mh in range(2):
                    ev = scr.tile([128, 512], FP, tag="ev", bufs=3, name="ev")
                    nc.vector.tensor_copy(out=ev[:, :cw], in_=P[ic2][mh][:, :cw])
                    nc.sync.dma_start(
                        out=dram_hvT[icp][
                            mh * 128 : (mh + 1) * 128, cs0 : cs0 + cw
                        ],
                        in_=ev[:, :cw],
                    )
            nc.gpsimd.collective_compute(
                "AllReduce", ALU.add, replica_groups=QUAD_GROUPS,
                ins=[dram_hvT[icp].opt()], outs=[dram_hvT_red[icp].opt()],
            )

        # =========== USER-side pass 1 ===========
        # (b_fac emitted here so its DVE ops don't block the item-side mask
        #  stream in the strict-FIFO DVE queue while the coldeg AR is in
        #  flight)
        b_fac = rsqrt_tiles(lambda s, p: dram_cd_red[:, s : s + p], IPT, "b")
        # HuT[m, u] partial = sum_r sum_i (b_i * maskT_r[i,u]) * Wi[r][m,i]
        P = [
            [ps_mm.tile([128, 500], FP, tag="p1", bufs=4, name="P") for _ in range(2)]
            for _ in range(2)
        ]
        for r in range(R):
            for kt, (isrt, pi) in enumerate(IPT):
                msk = scr.tile([128, 1000], BF, tag="mask", bufs=3, name="msk")
                nc.vector.tensor_scalar(
                    out=msk[:pi, :], in0=adjT[kt][:pi, :],
                    scalar1=float(r + 1), scalar2=b_fac[kt][:pi, :],
                    op0=ALU.is_equal, op1=ALU.mult,
                )
                first = r == 0 and kt == 0
                last = r == R - 1 and kt == len(IPT) - 1
                for mh in range(2):
                    for uc in range(2):
                        nc.tensor.matmul(
                            P[uc][mh][:, :],
                            lhsT=wiT[r][kt][:pi, mh * 128 : (mh + 1) * 128],
                            rhs=msk[:pi, uc * 500 : uc * 500 + 500],
                            start=first, stop=last,
                        )
        for uc in range(2):
            for mh in range(2):
                ev = scr.tile([128, 500], FP, tag="ev", bufs=3, name="ev")
                nc.vector.tensor_copy(out=ev[:, :], in_=P[uc][mh][:, :])
                nc.sync.dma_start(
                    out=dram_huT[mh * 128 : (mh + 1) * 128, uc * 500 : uc * 500 + 500],
                    in_=ev[:, :],
                )
        nc.gpsimd.collective_compute(
            "AllReduce", ALU.add, replica_groups=PAIR_GROUPS,
            ins=[dram_huT.opt()], outs=[dram_huT_red.opt()],
        )

        # release MM PSUM pool, open pass-2 pool
        ps_mm.release()
        ps_p2 = ctx.enter_context(tc.tile_pool(name="ps_p2", bufs=2, space="PSUM"))

        # ===== Pass-2 small-weight + side-feature prep (tail; uses PE) =====
        def load_t_small(w_dram, rows, cols, nm):
            f = scr.tile([128, 128], FP, tag="smf", name="smf")
            nc.sync.dma_start(out=f[:rows, :cols], in_=w_dram[:, :])
            bmat = scr.tile([128, 128], BF, tag="smb", name="smb")
            nc.scalar.copy(out=bmat[:rows, :cols], in_=f[:rows, :cols])
            pt_ps = ps_p2.tile([128, 1024], BF, tag="trp2", name="pt_ps")
            nc.tensor.transpose(
                pt_ps[:cols, :rows], bmat[:rows, :cols], ident[:rows, :rows]
            )
            outt = res.tile([128, max(rows, 8)], BF, tag=f"smT{nm}", name="outt")
            nc.scalar.copy(out=outt[:cols, :rows], in_=pt_ps[:cols, :rows])
            return outt

        dwT = []  # dense_W^T as two [128, OUT] tiles
        for mh in range(2):
            f = scr.tile([128, 128], FP, tag="smf", name="smf")
            nc.sync.dma_start(out=f[:OUT, :128], in_=dw[:, mh * 128 : (mh + 1) * 128])
            bmat = scr.tile([128, 128], BF, tag="smb", name="smb")
            nc.scalar.copy(out=bmat[:OUT, :128], in_=f[:OUT, :128])
            pt_ps = ps_p2.tile([128, 1024], BF, tag="trp2", name="pt_ps")
            nc.tensor.transpose(pt_ps[:128, :OUT], bmat[:OUT, :128], ident[:OUT, :OUT])
            t = res.tile([128, OUT], BF, tag=f"dwT{mh}", name="t")
            nc.scalar.copy(out=t[:, :], in_=pt_ps[:128, :OUT])
            dwT.append(t)

        uw1T = load_t_small(uw1, SIDE, FDIM, "uw1")  # [FDIM, SIDE]
        uw2T = load_t_small(uw2, OUT, SIDE, "uw2")  # [SIDE, OUT]
        vw1T = load_t_small(vw1, SIDE, FDIM, "vw1")
        vw2T = load_t_small(vw2, OUT, SIDE, "vw2")
        ub1_t = res.tile([SIDE, 1], FP, tag="biasu")
        nc.sync.dma_start(out=ub1_t[:, :], in_=ub1[:, :])
        vb1_t = res.tile([SIDE, 1], FP, tag="biasv")
        nc.sync.dma_start(out=vb1_t[:, :], in_=vb1[:, :])

        # side-feature transposes: sfT = bf16(sideFeat)^T [FDIM, n]
        def prep_sfT(side_dram, tiles, n, nm):
            sfT = res.tile([128, n], BF, tag=f"sfT{nm}", name="sfT")
            for g in range(0, len(tiles), 8):
                pt_ps = ps_p2.tile([128, 1024], BF, tag="trp2", name="pt_ps")
                w = 0
                g0 = tiles[g][0]
                for t in range(g, min(g + 8, len(tiles))):
                    s, p = tiles[t]
                    f = scr.tile([128, FDIM], FP, tag="p2f", name="f")
                    nc.sync.dma_start(out=f[:p, :], in_=side_dram[s : s + p, :])
                    bmat = scr.tile([128, FDIM], BF, tag="p2b", name="bmat")
                    nc.scalar.copy(out=bmat[:p, :], in_=f[:p, :])
                    nc.tensor.transpose(
                        pt_ps[:FDIM, w : w + p], bmat[:p, :], ident[:p, :p]
                    )
                    w += p
                nc.scalar.copy(out=sfT[:FDIM, g0 : g0 + w], in_=pt_ps[:FDIM, :w])
            return sfT

        sfT_v = prep_sfT(vf, IPT, BI, "v")
        sfT_u = prep_sfT(uf, UPT, BU, "u")


        def pass2(h_red_parts, sfT, w1T, bias_t, w2T, fac, tiles, n, o_dram, nm):
            # F^T = relu(w1 @ sf^T + b)  [SIDE, n] bf16
            fT = res.tile([SIDE, n], BF, tag=f"fT{nm}", name="fT")
            for c in range(0, n, 500):
                pf = ps_p2.tile([SIDE, 500], FP, tag="pf", name="pf")
                nc.tensor.matmul(
                    pf[:, :], lhsT=w1T[:FDIM, :SIDE], rhs=sfT[:FDIM, c : c + 500],
                    start=True, stop=True,
                )
                nc.scalar.activation(
                    out=fT[:, c : c + 500], in_=pf[:, :], func=AF.Relu,
                    bias=bias_t[:, :],
                )
            # consume each reduced part as it lands
            for dtile, c0, w in h_red_parts:
                hT = []
                for mh in range(2):
                    hf = scr.tile([128, 1024], FP, tag="p2h", name="hf")
                    nc.sync.dma_start(
                        out=hf[:, :w], in_=dtile[mh * 128 : (mh + 1) * 128, :w]
                    )
                    hb = scr.tile([128, 1024], BF, tag="p2hb", bufs=4, name="hb")
                    nc.scalar.activation(out=hb[:, :w], in_=hf[:, :w], func=AF.Relu)
                    hT.append(hb)
                for t, (s, p) in enumerate(tiles):
                    if not (c0 <= s < c0 + w):
                        continue
                    sl = s - c0
                    pa = ps_p2.tile([128, OUT], FP, tag="pa", name="pa")
                    for mh in range(2):
                        nc.tensor.matmul(
                            pa[:p, :], lhsT=hT[mh][:, sl : sl + p],
                            rhs=dwT[mh][:, :OUT],
                            start=(mh == 0), stop=(mh == 1),
                        )
                    sa = scr.tile([128, OUT], FP, tag="p2sa", name="sa")
                    nc.scalar.activation(
                        out=sa[:p, :], in_=pa[:p, :], func=AF.Copy, scale=fac[t][:p, :]
                    )
                    pb = ps_p2.tile([128, OUT], FP, tag="pb", name="pb")
                    nc.tensor.matmul(
                        pb[:p, :], lhsT=fT[:SIDE, s : s + p], rhs=w2T[:SIDE, :OUT],
                        start=True, stop=True,
                    )
                    so = scr.tile([128, OUT], FP, tag="p2so", name="so")
                    nc.vector.tensor_tensor(
                        out=so[:p, :], in0=pb[:p, :], in1=sa[:p, :], op=ALU.add
                    )
                    ro = scr.tile([128, OUT], FP, tag="p2ro", name="ro")
                    nc.scalar.activation(out=ro[:p, :], in_=so[:p, :], func=AF.Relu)
                    nc.sync.dma_start(out=o_dram[s : s + p, :], in_=ro[:p, :])

        pass2(
            [(dram_hvT_red[0], 0, 1024), (dram_hvT_red[1], 1024, 976)],
            sfT_v, vw1T, vb1_t, vw2T, b_fac, IPT, BI, v_out, "v",
        )
        pass2(
            [(dram_huT_red, 0, 1000)],
            sfT_u, uw1T, ub1_t, uw2T, a_fac, UPT, BU, u_out, "u",
        )

    nc.compile()
    return nc


_CACHE = {}


def _get_program():
    if "nc" not in _CACHE:
        _CACHE["nc"] = build_program()
    return _CACHE["nc"]


def make_in_maps(inputs):
    adj = np.asarray(inputs["adj_matrix"], dtype=np.int32)
    u_sf = np.asarray(inputs["u_sideFeat"], dtype=np.float32)
    v_sf = np.asarray(inputs["v_sideFeat"], dtype=np.float32)
    msg_W = np.asarray(inputs["msg_W"], dtype=np.float32)
    dense_W = np.asarray(inputs["dense_W"], dtype=np.float32)
    u_W1 = np.asarray(inputs["u_W1"], dtype=np.float32)
    u_b1 = np.asarray(inputs["u_b1"], dtype=np.float32).reshape(SIDE, 1)
    u_W2 = np.asarray(inputs["u_W2"], dtype=np.float32)
    v_W1 = np.asarray(inputs["v_W1"], dtype=np.float32)
    v_b1 = np.asarray(inputs["v_b1"], dtype=np.float32).reshape(SIDE, 1)
    v_W2 = np.asarray(inputs["v_W2"], dtype=np.float32)

    in_maps = []
    for a in range(GA):
        for b in range(GB):
            in_maps.append(
                {
                    "adj_blk": np.ascontiguousarray(
                        adj[a * BU : (a + 1) * BU, b * BI : (b + 1) * BI]
                    ),
                    # pre-transposed W slices: [R, n, M]
                    "wi": np.ascontiguousarray(
                        msg_W[:, :, NU + b * BI : NU + (b + 1) * BI].transpose(0, 2, 1)
                    ),
                    "wu": np.ascontiguousarray(
                        msg_W[:, :, a * BU : (a + 1) * BU].transpose(0, 2, 1)
                    ),
                    "uf": np.ascontiguousarray(u_sf[a * BU : (a + 1) * BU]),
                    "vf": np.ascontiguousarray(v_sf[b * BI : (b + 1) * BI]),
                    "dw": dense_W,
                    "uw1": u_W1,
                    "ub1": u_b1,
                    "uw2": u_W2,
                    "vw1": v_W1,
                    "vb1": v_b1,
                    "vw2": v_W2,
                }
            )
    return in_maps


def assemble(results):
    U = np.empty((NU, OUT), np.float32)
    V = np.empty((NI, OUT), np.float32)
    for a in range(GA):
        U[a * BU : (a + 1) * BU] = results[a * GB]["u_out"]
    for b in range(GB):
        V[b * BI : (b + 1) * BI] = results[b]["v_out"]
    return (U, V)


def kernel(**inputs):
    from concourse.bass_utils import run_bass_kernel_spmd

    nc = _get_program()
    res = run_bass_kernel_spmd(nc, make_in_maps(inputs), core_ids=list(range(NCORES)))
    return assemble(res.results)



# revision 4
# speedup vs baseline: 1.4761x; 1.4761x over previous
"""Trainium2 Bass kernel for the bipartite GNN message-passing encoder.

Math (see reference.py):
  A_r = (adj == r), r = 1..5
  An_r = diag(1/sqrt(Nu)) A_r diag(1/sqrt(Nv))
  Hu = relu(sum_r An_r @ W_items_r^T)   [NU, M]
  Hv = relu(sum_r An_r^T @ W_users_r^T) [NI, M]
  U  = relu(Hu @ dense_W^T + relu(u_sideFeat @ u_W1^T + u_b1) @ u_W2^T)
  V  = relu(Hv @ dense_W^T + relu(v_sideFeat @ v_W1^T + v_b1) @ v_W2^T)

Sharding: symmetric 1D. Core c owns users U_c = [500c, 500c+500) and items
I_c = [500c, 500c+500). The host hands each core TWO adjacency views in
bf16 (values 0..5 are exact): adjR = adj[U_c, :] (full rows) and
adjCT = adj[:, I_c]^T (full columns, pre-transposed). Row degrees for U_c
and column degrees for I_c are therefore LOCAL - no degree collectives -
so the pass-1 mask-matmul streams start within ~7us of launch. Each
stream produces a partial over the full opposite side (HvT partial
[M, NI] from my users; HuT partial [M, NU] from my items), laid out in
DRAM blocked by destination core [8, M, 500] and combined with a single
bf16 ReduceScatter each. Pass 2 is then fully local.

A 4-byte dummy AllReduce is triggered as the first instruction so the
collectives init barrier (which waits for the slowest core; ~125us of
launch skew measured on this fabric) overlaps the local compute instead
of delaying the first real collective.

All weight/side tensors arrive host-pre-transposed and bf16-converted:
the PE runs matmuls only (no on-device transposes), DVE builds masks
(dual-op is_equal x per-partition degree factor) and half the PSUM
evacuations, ACT does the rest. Degree factors for tile kt are emitted
just before tile kt's first mask so the DVE never runs ahead of the PE.
"""

import sys

import numpy as np

if "/opt/trn_rl_repo" not in sys.path:
    sys.path.insert(0, "/opt/trn_rl_repo")

import concourse.bacc as bacc  # noqa: E402
import concourse.mybir as mybir  # noqa: E402
import concourse.tile as tile  # noqa: E402

FP = mybir.dt.float32
BF = mybir.dt.bfloat16

NU = NI = 4000
R = 5
M = 256
OUT = 75
SIDE = 64
FDIM = 128

NCORES = 8
BU = NU // NCORES  # 500 users per core
BI = NI // NCORES  # 500 items per core

AF = mybir.ActivationFunctionType
ALU = mybir.AluOpType

ALL_GROUP = [list(range(NCORES))]
PAIR_GROUPS = [[2 * a, 2 * a + 1] for a in range(NCORES // 2)]

PT = [(t * 125, 125) for t in range(4)]  # 4 partition tiles over 500


def build_program():
    from contextlib import ExitStack

    nc = bacc.Bacc("TRN2", target_bir_lowering=False, debug=False, num_devices=NCORES)

    # ---- I/O ---- (all host-sliced / pre-transposed; bf16 except biases)
    adjR = nc.dram_tensor("adjR", [BU, NI], BF, kind="ExternalInput")
    adjCT = nc.dram_tensor("adjCT", [BI, NU], BF, kind="ExternalInput")
    wu = nc.dram_tensor("wu", [R, BU, M], BF, kind="ExternalInput")
    wi = nc.dram_tensor("wi", [R, BI, M], BF, kind="ExternalInput")
    ufT = nc.dram_tensor("ufT", [FDIM, BU], BF, kind="ExternalInput")
    vfT = nc.dram_tensor("vfT", [FDIM, BI], BF, kind="ExternalInput")
    dwT = nc.dram_tensor("dwT", [M, OUT], BF, kind="ExternalInput")
    uw1T = nc.dram_tensor("uw1T", [FDIM, SIDE], BF, kind="ExternalInput")
    ub1 = nc.dram_tensor("ub1", [SIDE, 1], FP, kind="ExternalInput")
    uw2T = nc.dram_tensor("uw2T", [SIDE, OUT], BF, kind="ExternalInput")
    vw1T = nc.dram_tensor("vw1T", [FDIM, SIDE], BF, kind="ExternalInput")
    vb1 = nc.dram_tensor("vb1", [SIDE, 1], FP, kind="ExternalInput")
    vw2T = nc.dram_tensor("vw2T", [SIDE, OUT], BF, kind="ExternalInput")
    u_out = nc.dram_tensor("u_out", [BU, OUT], FP, kind="ExternalOutput")
    v_out = nc.dram_tensor("v_out", [BI, OUT], FP, kind="ExternalOutput")

    with tile.TileContext(nc) as tc, ExitStack() as ctx:
        res = ctx.enter_context(tc.tile_pool(name="res", bufs=1))
        scr = ctx.enter_context(tc.tile_pool(name="scr", bufs=2))
        dram = ctx.enter_context(tc.tile_pool(name="dram", bufs=1, space="DRAM"))

        # ---- dummy collective: absorbs the init barrier during compute ----
        dummy_src = res.tile([1, 8], FP, tag="dummy_src")
        nc.gpsimd.memset(dummy_src[:], 0.0)
        dram_dmy = dram.tile([1, 8], FP, tag="dram_dmy")
        dram_dmy_o = dram.tile([1, 8], FP, tag="dram_dmy_o")
        nc.sync.dma_start(out=dram_dmy[:, :], in_=dummy_src[:, :])
        nc.gpsimd.collective_compute(
            "AllReduce", ALU.add, replica_groups=PAIR_GROUPS,
            ins=[dram_dmy.opt()], outs=[dram_dmy_o.opt()],
        )

        # ---- adjacency DMAs on the sync queue (big, first) ----
        def load_adj(a_dram, nm):
            at = []
            for kt, (s, p) in enumerate(PT):
                ab = res.tile([128, NI], BF, tag=f"{nm}{kt}", name="ab")
                nc.sync.dma_start(out=ab[:p, :], in_=a_dram[s : s + p, :])
                at.append(ab)
            return at

        adjR_t = load_adj(adjR, "aR")
        adjCT_t = load_adj(adjCT, "aC")

        # ---- weight DMAs on the scalar queue (parallel with adj) ----
        def load_w(w_dram, nm):
            out = [[None] * 4 for _ in range(R)]
            for r in range(R):
                for kt, (s, p) in enumerate(PT):
                    wt = res.tile([128, M], BF, tag=f"{nm}{r}_{kt}", name="wt")
                    nc.scalar.dma_start(out=wt[:p, :], in_=w_dram[r, s : s + p, :])
                    out[r][kt] = wt
            return out

        wuT = load_w(wu, "wu")
        wiT = load_w(wi, "wi")

        # ---- local degree factors: emitted lazily per tile inside pass 1 ----
        a_fac = [None] * 4
        b_fac = [None] * 4

        def emit_deg(adj_t, fac, kt, nm):
            p = PT[kt][1]
            nz = scr.tile([128, NI], BF, tag="nz", bufs=2, name="nz")
            nc.vector.tensor_scalar(
                out=nz[:p, :], in0=adj_t[kt][:p, :], scalar1=1.0, scalar2=None,
                op0=ALU.min,
            )
            dg = scr.tile([128, 1], FP, tag="dg", bufs=2, name="dg")
            nc.vector.tensor_reduce(
                out=dg[:p, :], in_=nz[:p, :], axis=mybir.AxisListType.X, op=ALU.add,
            )
            m1 = scr.tile([128, 1], FP, tag="m1", bufs=2, name="m1")
            nc.vector.tensor_scalar(
                out=m1[:p, :], in0=dg[:p, :], scalar1=1.0, scalar2=None, op0=ALU.max,
            )
            sq = scr.tile([128, 1], FP, tag="sq", bufs=2, name="sq")
            nc.scalar.sqrt(out=sq[:p, :], in_=m1[:p, :])
            fc = res.tile([128, 1], FP, tag=f"{nm}fac{kt}", name="fc")
            nc.vector.reciprocal(out=fc[:p, :], in_=sq[:p, :])
            fac[kt] = fc

        ps_mm = tc.alloc_tile_pool(name="ps_mm", bufs=1, space="PSUM")

        # DRAM partial buffers, blocked by destination core [8, M, 500]
        dram_hv = dram.tile([NCORES, M, BI], BF, tag="dram_hv")
        dram_hu = dram.tile([NCORES, M, BU], BF, tag="dram_hu")
        dram_hv_red = dram.tile([M, BI], BF, tag="dram_hv_red")
        dram_hu_red = dram.tile([M, BU], BF, tag="dram_hu_red")

        # ---- pass 1: one side = 2 halves x (4kt x 5r masks -> 8-bank matmul) ----
        def pass1(adj_t, fac, wT, w_blk, dram_part, prep):
            # partial H^T[m, col] = sum_r sum_p (fac_p * mask_r[p, col]) * W[r][m, p]
            for h in range(2):
                P = [
                    [
                        ps_mm.tile([128, w_blk], FP, tag=f"p{mh}{cc}", name="P")
                        for cc in range(4)
                    ]
                    for mh in range(2)
                ]
                for kt, (s, p) in enumerate(PT):
                    if prep is not None:
                        prep(h, kt)
                    for r in range(R):
                        msk = scr.tile(
                            [128, 4 * w_blk], BF, tag="mask", bufs=3, name="msk"
                        )
                        nc.vector.tensor_scalar(
                            out=msk[:p, :],
                            in0=adj_t[kt][:p, h * 4 * w_blk : (h + 1) * 4 * w_blk],
                            scalar1=float(r + 1), scalar2=fac[kt][:p, :],
                            op0=ALU.is_equal, op1=ALU.mult,
                        )
                        first = kt == 0 and r == 0
                        last = kt == 3 and r == R - 1
                        for mh in range(2):
                            for cc in range(4):
                                nc.tensor.matmul(
                                    P[mh][cc][:, :],
                                    lhsT=wT[r][kt][:p, mh * 128 : (mh + 1) * 128],
                                    rhs=msk[:p, cc * w_blk : (cc + 1) * w_blk],
                                    start=first, stop=last,
                                )
                # evacuate in matmul emission order so the next half's first
                # matmul only waits on its own bank; alternate ACT/DVE
                for mh in range(2):
                    for cc in range(4):
                        ev = scr.tile([128, w_blk], BF, tag="ev", bufs=4, name="ev")
                        if cc % 2 == 0:
                            nc.scalar.copy(out=ev[:, :], in_=P[mh][cc][:, :])
                        else:
                            nc.vector.tensor_copy(out=ev[:, :], in_=P[mh][cc][:, :])
                        nc.sync.dma_start(
                            out=dram_part[h * 4 + cc, mh * 128 : (mh + 1) * 128, :],
                            in_=ev[:, :],
                        )

        def item_prep(h, kt):
            if h == 0:
                emit_deg(adjR_t, a_fac, kt, "a")
            else:
                emit_deg(adjCT_t, b_fac, kt, "b")

        pass1(adjR_t, a_fac, wuT, BI, dram_hv, item_prep)
        nc.gpsimd.collective_compute(
            "ReduceScatter", ALU.add, replica_groups=ALL_GROUP,
            ins=[dram_hv.opt()], outs=[dram_hv_red.opt()],
        )
        pass1(adjCT_t, b_fac, wiT, BU, dram_hu, None)
        nc.gpsimd.collective_compute(
            "ReduceScatter", ALU.add, replica_groups=ALL_GROUP,
            ins=[dram_hu.opt()], outs=[dram_hu_red.opt()],
        )

        ps_mm.release()
        ps_p2 = ctx.enter_context(tc.tile_pool(name="ps_p2", bufs=2, space="PSUM"))

        # ---- side-feature heads (PE is free once pass 1 drains) ----
        def side_head(w1T_d, b_d, sfT_d, n, nm):
            w1t = scr.tile([128, SIDE], BF, tag="w1t", name="w1t")
            nc.scalar.dma_start(out=w1t[:FDIM, :], in_=w1T_d[:, :])
            bia = res.tile([SIDE, 1], FP, tag=f"b_{nm}", name="bia")
            nc.scalar.dma_start(out=bia[:, :], in_=b_d[:, :])
            sft = scr.tile([128, n], BF, tag="sft", name="sft")
            nc.scalar.dma_start(out=sft[:FDIM, :], in_=sfT_d[:, :])
            fT = res.tile([SIDE, n], BF, tag=f"fT_{nm}", name="fT")
            pf = ps_p2.tile([SIDE, n], FP, tag="pf", name="pf")
            nc.tensor.matmul(
                pf[:, :], lhsT=w1t[:FDIM, :SIDE], rhs=sft[:FDIM, :],
                start=True, stop=True,
            )
            nc.scalar.activation(
                out=fT[:, :], in_=pf[:, :], func=AF.Relu, bias=bia[:, :],
            )
            return fT

        fT_v = side_head(vw1T, vb1, vfT, BI, "v")
        fT_u = side_head(uw1T, ub1, ufT, BU, "u")

        dwT_sb = []
        for mh in range(2):
            t = res.tile([128, OUT], BF, tag=f"dwT{mh}", name="t")
            nc.scalar.dma_start(out=t[:, :], in_=dwT[mh * 128 : (mh + 1) * 128, :])
            dwT_sb.append(t)
        uw2_sb = res.tile([SIDE, OUT], BF, tag="uw2")
        nc.scalar.dma_start(out=uw2_sb[:, :], in_=uw2T[:, :])
        vw2_sb = res.tile([SIDE, OUT], BF, tag="vw2")
        nc.scalar.dma_start(out=vw2_sb[:, :], in_=vw2T[:, :])

        # ---- pass 2 (fully local): out = relu(fac*relu(H)@dW^T + F@W2^T) ----
        def pass2(h_red, fT, w2_sb, fac, n, o_dram, nm):
            hT = []
            for mh in range(2):
                hf = scr.tile([128, n], BF, tag="p2h", bufs=4, name="hf")
                nc.sync.dma_start(
                    out=hf[:, :], in_=h_red[mh * 128 : (mh + 1) * 128, :]
                )
                hb = scr.tile([128, n], BF, tag="p2hb", bufs=4, name="hb")
                nc.scalar.activation(out=hb[:, :], in_=hf[:, :], func=AF.Relu)
                hT.append(hb)
            for kt, (s, p) in enumerate(PT):
                pa = ps_p2.tile([128, OUT], FP, tag="pa", name="pa")
                for mh in range(2):
                    nc.tensor.matmul(
                        pa[:p, :], lhsT=hT[mh][:, s : s + p], rhs=dwT_sb[mh][:, :OUT],
                        start=(mh == 0), stop=(mh == 1),
                    )
                sa = scr.tile([128, OUT], FP, tag="p2sa", name="sa")
                nc.scalar.activation(
                    out=sa[:p, :], in_=pa[:p, :], func=AF.Copy, scale=fac[kt][:p, :]
                )
                pb = ps_p2.tile([128, OUT], FP, tag="pb", name="pb")
                nc.tensor.matmul(
                    pb[:p, :], lhsT=fT[:SIDE, s : s + p], rhs=w2_sb[:SIDE, :OUT],
                    start=True, stop=True,
                )
                so = scr.tile([128, OUT], FP, tag="p2so", name="so")
                nc.vector.tensor_tensor(
                    out=so[:p, :], in0=pb[:p, :], in1=sa[:p, :], op=ALU.add
                )
                ro = scr.tile([128, OUT], FP, tag="p2ro", name="ro")
                nc.scalar.activation(out=ro[:p, :], in_=so[:p, :], func=AF.Relu)
                nc.sync.dma_start(out=o_dram[s : s + p, :], in_=ro[:p, :])

        pass2(dram_hv_red, fT_v, vw2_sb, b_fac, BI, v_out, "v")
        pass2(dram_hu_red, fT_u, uw2_sb, a_fac, BU, u_out, "u")

    nc.compile()
    return nc


_CACHE = {}


def _get_program():
    if "nc" not in _CACHE:
        _CACHE["nc"] = build_program()
    return _CACHE["nc"]


def make_in_maps(inputs):
    import ml_dtypes

    bf = ml_dtypes.bfloat16
    adj = np.asarray(inputs["adj_matrix"], dtype=np.int32)
    adjB = adj.astype(bf)  # values 0..5: exact in bf16
    msg_W = np.asarray(inputs["msg_W"], np.float32).astype(bf)
    u_sfT = np.asarray(inputs["u_sideFeat"], np.float32).astype(bf).T
    v_sfT = np.asarray(inputs["v_sideFeat"], np.float32).astype(bf).T
    dwT = np.ascontiguousarray(np.asarray(inputs["dense_W"], np.float32).astype(bf).T)
    uw1T = np.ascontiguousarray(np.asarray(inputs["u_W1"], np.float32).astype(bf).T)
    ub1 = np.asarray(inputs["u_b1"], np.float32).reshape(SIDE, 1)
    uw2T = np.ascontiguousarray(np.asarray(inputs["u_W2"], np.float32).astype(bf).T)
    vw1T = np.ascontiguousarray(np.asarray(inputs["v_W1"], np.float32).astype(bf).T)
    vb1 = np.asarray(inputs["v_b1"], np.float32).reshape(SIDE, 1)
    vw2T = np.ascontiguousarray(np.asarray(inputs["v_W2"], np.float32).astype(bf).T)

    in_maps = []
    for c in range(NCORES):
        us, ie = c * BU, c * BI
        in_maps.append(
            {
                "adjR": np.ascontiguousarray(adjB[us : us + BU, :]),
                "adjCT": np.ascontiguousarray(adjB[:, ie : ie + BI].T),
                # pre-transposed W slices: [R, n, M]
                "wu": np.ascontiguousarray(
                    msg_W[:, :, us : us + BU].transpose(0, 2, 1)
                ),
                "wi": np.ascontiguousarray(
                    msg_W[:, :, NU + ie : NU + ie + BI].transpose(0, 2, 1)
                ),
                "ufT": np.ascontiguousarray(u_sfT[:, us : us + BU]),
                "vfT": np.ascontiguousarray(v_sfT[:, ie : ie + BI]),
                "dwT": dwT,
                "uw1T": uw1T,
                "ub1": ub1,
                "uw2T": uw2T,
                "vw1T": vw1T,
                "vb1": vb1,
                "vw2T": vw2T,
            }
        )
    return in_maps


def assemble(results):
    U = np.empty((NU, OUT), np.float32)
    V = np.empty((NI, OUT), np.float32)
    for c in range(NCORES):
        U[c * BU : (c + 1) * BU] = results[c]["u_out"]
        V[c * BI : (c + 1) * BI] = results[c]["v_out"]
    return (U, V)


def kernel(**inputs):
    from concourse.bass_utils import run_bass_kernel_spmd

    nc = _get_program()
    res = run_bass_kernel_spmd(nc, make_in_maps(inputs), core_ids=list(range(NCORES)))
    return assemble(res.results)


# revision 6
# speedup vs baseline: 1.6049x; 1.0873x over previous
"""Trainium2 Bass kernel for the bipartite GNN message-passing encoder.

Math (see reference.py):
  A_r = (adj == r), r = 1..5
  An_r = diag(1/sqrt(Nu)) A_r diag(1/sqrt(Nv))
  Hu = relu(sum_r An_r @ W_items_r^T)   [NU, M]
  Hv = relu(sum_r An_r^T @ W_users_r^T) [NI, M]
  U  = relu(Hu @ dense_W^T + relu(u_sideFeat @ u_W1^T + u_b1) @ u_W2^T)
  V  = relu(Hv @ dense_W^T + relu(v_sideFeat @ v_W1^T + v_b1) @ v_W2^T)

Sharding: symmetric 1D. Core c owns users U_c = [500c, 500c+500) and items
I_c = [500c, 500c+500). The host hands each core TWO adjacency views in
bf16 (values 0..5 are exact): adjR = adj[U_c, :] (full rows) and
adjCT = adj[:, I_c]^T (full columns, pre-transposed). Row degrees for U_c
and column degrees for I_c are therefore LOCAL - no degree collectives -
so the pass-1 mask-matmul streams start within ~7us of launch. Each
stream produces a partial over the full opposite side (HvT partial
[M, NI] from my users; HuT partial [M, NU] from my items), laid out in
DRAM blocked by destination core [8, M, 500] and combined with a single
bf16 ReduceScatter each. Pass 2 is then fully local.

A 4-byte dummy AllReduce is triggered as the first instruction so the
collectives init barrier (which waits for the slowest core's trigger)
overlaps the local compute instead of delaying the first real
collective.

Engine budget: PE runs 640 back-to-back [<=125c x 128 x 500] matmuls
(~133us, no transposes - everything arrives host-pre-transposed); DVE
builds the 80 masks (dual-op is_equal x per-partition degree factor,
~1us each) plus the fused degree rows (accum_out); ACT evacuates PSUM
and does pass-2 activations. DMA issue cost is real (~0.6us/issue on
sync, ~2us on scalar), so all weights are host-packed into a handful of
wide tensors loaded by single DMAs, ordered so the kt=0 operands land
first.
"""

import sys

import numpy as np

if "/opt/trn_rl_repo" not in sys.path:
    sys.path.insert(0, "/opt/trn_rl_repo")

import concourse.bacc as bacc  # noqa: E402
import concourse.mybir as mybir  # noqa: E402
import concourse.tile as tile  # noqa: E402

FP = mybir.dt.float32
BF = mybir.dt.bfloat16

NU = NI = 4000
R = 5
M = 256
OUT = 75
SIDE = 64
FDIM = 128

NCORES = 8
BU = NU // NCORES  # 500 users per core
BI = NI // NCORES  # 500 items per core

AF = mybir.ActivationFunctionType
ALU = mybir.AluOpType

ALL_GROUP = [list(range(NCORES))]
PAIR_GROUPS = [[2 * a, 2 * a + 1] for a in range(NCORES // 2)]

PT = [(t * 125, 125) for t in range(4)]  # 4 partition tiles over 500
WCOLS = R * 4 * M  # 5120 packed weight columns
# smallpack column layout
SP_DW = 0  # [128, 2x75] dense_W^T halves
SP_UW1 = 150  # [128, 64]
SP_VW1 = 214  # [128, 64]
SP_UW2 = 278  # [64, 75]
SP_VW2 = 353  # [64, 75]
SP_COLS = 428


def build_program():
    from contextlib import ExitStack

    nc = bacc.Bacc("TRN2", target_bir_lowering=False, debug=False, num_devices=NCORES)

    # ---- I/O ---- (all host-sliced / packed / pre-transposed bf16)
    adjR = nc.dram_tensor("adjR", [BU, NI], BF, kind="ExternalInput")
    adjCT = nc.dram_tensor("adjCT", [BI, NU], BF, kind="ExternalInput")
    wuH = nc.dram_tensor("wuH", [125, WCOLS], BF, kind="ExternalInput")
    wiH = nc.dram_tensor("wiH", [125, WCOLS], BF, kind="ExternalInput")
    ufT = nc.dram_tensor("ufT", [FDIM, BU], BF, kind="ExternalInput")
    vfT = nc.dram_tensor("vfT", [FDIM, BI], BF, kind="ExternalInput")
    smallpack = nc.dram_tensor("smallpack", [128, SP_COLS], BF, kind="ExternalInput")
    ub1 = nc.dram_tensor("ub1", [SIDE, 1], FP, kind="ExternalInput")
    vb1 = nc.dram_tensor("vb1", [SIDE, 1], FP, kind="ExternalInput")
    u_out = nc.dram_tensor("u_out", [BU, OUT], FP, kind="ExternalOutput")
    v_out = nc.dram_tensor("v_out", [BI, OUT], FP, kind="ExternalOutput")

    with tile.TileContext(nc) as tc, ExitStack() as ctx:
        res = ctx.enter_context(tc.tile_pool(name="res", bufs=1))
        scr = ctx.enter_context(tc.tile_pool(name="scr", bufs=2))
        dram = ctx.enter_context(tc.tile_pool(name="dram", bufs=1, space="DRAM"))

        # ---- dummy collective: absorbs the init barrier during compute ----
        dummy_src = res.tile([1, 8], FP, tag="dummy_src")
        nc.gpsimd.memset(dummy_src[:], 0.0)
        dram_dmy = dram.tile([1, 8], FP, tag="dram_dmy")
        dram_dmy_o = dram.tile([1, 8], FP, tag="dram_dmy_o")
        nc.sync.dma_start(out=dram_dmy[:, :], in_=dummy_src[:, :])
        nc.gpsimd.collective_compute(
            "AllReduce", ALU.add, replica_groups=PAIR_GROUPS,
            ins=[dram_dmy.opt()], outs=[dram_dmy_o.opt()],
        )

        # ---- input DMAs, ordered so kt=0 operands land first ----
        adjR_t, adjCT_t = [], []
        ab0 = res.tile([128, NI], BF, tag="aR0", name="ab0")
        nc.sync.dma_start(out=ab0[:125, :], in_=adjR[0:125, :])
        adjR_t.append(ab0)
        wu_sb = res.tile([125, WCOLS], BF, tag="wu_sb")
        nc.sync.dma_start(out=wu_sb[:, :], in_=wuH[:, :])
        for kt, (s, p) in enumerate(PT):
            if kt == 0:
                continue
            ab = res.tile([128, NI], BF, tag=f"aR{kt}", name="ab")
            nc.sync.dma_start(out=ab[:p, :], in_=adjR[s : s + p, :])
            adjR_t.append(ab)
        for kt, (s, p) in enumerate(PT):
            ac = res.tile([128, NU], BF, tag=f"aC{kt}", name="ac")
            nc.sync.dma_start(out=ac[:p, :], in_=adjCT[s : s + p, :])
            adjCT_t.append(ac)
        wi_sb = res.tile([125, WCOLS], BF, tag="wi_sb")
        nc.sync.dma_start(out=wi_sb[:, :], in_=wiH[:, :])
        ufT_sb = res.tile([128, BU], BF, tag="ufT_sb")
        nc.sync.dma_start(out=ufT_sb[:, :], in_=ufT[:, :])
        vfT_sb = res.tile([128, BI], BF, tag="vfT_sb")
        nc.sync.dma_start(out=vfT_sb[:, :], in_=vfT[:, :])
        sp_sb = res.tile([128, SP_COLS], BF, tag="sp_sb")
        nc.sync.dma_start(out=sp_sb[:, :], in_=smallpack[:, :])
        ub1_t = res.tile([SIDE, 1], FP, tag="ub1_t")
        nc.sync.dma_start(out=ub1_t[:, :], in_=ub1[:, :])
        vb1_t = res.tile([SIDE, 1], FP, tag="vb1_t")
        nc.sync.dma_start(out=vb1_t[:, :], in_=vb1[:, :])

        def wsl(w_sb, r, kt, mh):  # packed lhsT slice [125, 128]
            c = (r * 4 + kt) * M + mh * 128
            return w_sb[:125, c : c + 128]

        # ---- local degree factors: fused nz+rowsum, emitted lazily ----
        a_fac = [None] * 4
        b_fac = [None] * 4

        def emit_deg(adj_t, fac, kt, nm):
            p = PT[kt][1]
            nz = scr.tile([128, NI], BF, tag="nz", bufs=2, name="nz")
            dg = scr.tile([128, 1], FP, tag="dg", bufs=2, name="dg")
            nc.vector.tensor_scalar(
                out=nz[:p, :], in0=adj_t[kt][:p, :], scalar1=1.0, scalar2=0.0,
                op0=ALU.min, op1=ALU.add, accum_out=dg[:p, :],
            )
            m1 = scr.tile([128, 1], FP, tag="m1", bufs=2, name="m1")
            nc.vector.tensor_scalar(
                out=m1[:p, :], in0=dg[:p, :], scalar1=1.0, scalar2=None, op0=ALU.max,
            )
            sq = scr.tile([128, 1], FP, tag="sq", bufs=2, name="sq")
            nc.scalar.sqrt(out=sq[:p, :], in_=m1[:p, :])
            fc = res.tile([128, 1], FP, tag=f"{nm}fac{kt}", name="fc")
            nc.vector.reciprocal(out=fc[:p, :], in_=sq[:p, :])
            fac[kt] = fc

        ps_mm = tc.alloc_tile_pool(name="ps_mm", bufs=1, space="PSUM")

        # DRAM partial buffers, blocked by destination core [8, M, 500]
        dram_hv = dram.tile([NCORES, M, BI], BF, tag="dram_hv")
        dram_hu = dram.tile([NCORES, M, BU], BF, tag="dram_hu")
        dram_hv_red = dram.tile([M, BI], BF, tag="dram_hv_red")
        dram_hu_red = dram.tile([M, BU], BF, tag="dram_hu_red")

        # ---- pass 1: one side = 2 halves x (4kt x 5r masks -> 8-bank matmul) ----
        def pass1(adj_t, fac, w_sb, w_blk, dram_part, prep):
            # partial H^T[m, col] = sum_r sum_p (fac_p * mask_r[p, col]) * W[r][m, p]
            for h in range(2):
                P = [
                    [
                        ps_mm.tile([128, w_blk], FP, tag=f"p{mh}{cc}", name="P")
                        for cc in range(4)
                    ]
                    for mh in range(2)
                ]
                for kt, (s, p) in enumerate(PT):
                    if prep is not None:
                        prep(h, kt)
                    for r in range(R):
                        msk = scr.tile(
                            [128, 4 * w_blk], BF, tag="mask", bufs=3, name="msk"
                        )
                        nc.vector.tensor_scalar(
                            out=msk[:p, :],
                            in0=adj_t[kt][:p, h * 4 * w_blk : (h + 1) * 4 * w_blk],
                            scalar1=float(r + 1), scalar2=fac[kt][:p, :],
                            op0=ALU.is_equal, op1=ALU.mult,
                        )
                        first = kt == 0 and r == 0
                        last = kt == 3 and r == R - 1
                        for mh in range(2):
                            for cc in range(4):
                                nc.tensor.matmul(
                                    P[mh][cc][:, :],
                                    lhsT=wsl(w_sb, r, kt, mh),
                                    rhs=msk[:p, cc * w_blk : (cc + 1) * w_blk],
                                    start=first, stop=last,
                                )
                # evacuate in matmul emission order so the next half's first
                # matmul only waits on its own bank
                for mh in range(2):
                    for cc in range(4):
                        ev = scr.tile([128, w_blk], BF, tag="ev", bufs=4, name="ev")
                        nc.scalar.copy(out=ev[:, :], in_=P[mh][cc][:, :])
                        nc.sync.dma_start(
                            out=dram_part[h * 4 + cc, mh * 128 : (mh + 1) * 128, :],
                            in_=ev[:, :],
                        )

        def item_prep(h, kt):
            if h == 0:
                emit_deg(adjR_t, a_fac, kt, "a")
            else:
                emit_deg(adjCT_t, b_fac, kt, "b")

        pass1(adjR_t, a_fac, wu_sb, BI, dram_hv, item_prep)
        nc.gpsimd.collective_compute(
            "ReduceScatter", ALU.add, replica_groups=ALL_GROUP,
            ins=[dram_hv.opt()], outs=[dram_hv_red.opt()],
        )
        pass1(adjCT_t, b_fac, wi_sb, BU, dram_hu, None)
        nc.gpsimd.collective_compute(
            "ReduceScatter", ALU.add, replica_groups=ALL_GROUP,
            ins=[dram_hu.opt()], outs=[dram_hu_red.opt()],
        )

        ps_mm.release()
        ps_p2 = ctx.enter_context(tc.tile_pool(name="ps_p2", bufs=2, space="PSUM"))

        # ---- side-feature heads (PE is free once pass 1 drains) ----
        def side_head(w1c, bia, sft, n, nm):
            fT = res.tile([SIDE, n], BF, tag=f"fT_{nm}", name="fT")
            pf = ps_p2.tile([SIDE, n], FP, tag="pf", name="pf")
            nc.tensor.matmul(
                pf[:, :], lhsT=sp_sb[:FDIM, w1c : w1c + SIDE], rhs=sft[:FDIM, :],
                start=True, stop=True,
            )
            nc.scalar.activation(
                out=fT[:, :], in_=pf[:, :], func=AF.Relu, bias=bia[:, :],
            )
            return fT

        fT_v = side_head(SP_VW1, vb1_t, vfT_sb, BI, "v")
        fT_u = side_head(SP_UW1, ub1_t, ufT_sb, BU, "u")

        # ---- pass 2 (fully local): out = relu(fac*relu(H)@dW^T + F@W2^T) ----
        def pass2(h_red, fT, w2c, fac, n, o_dram, nm):
            hT = []
            for mh in range(2):
                hf = scr.tile([128, n], BF, tag="p2h", bufs=4, name="hf")
                nc.sync.dma_start(
                    out=hf[:, :], in_=h_red[mh * 128 : (mh + 1) * 128, :]
                )
                hb = scr.tile([128, n], BF, tag="p2hb", bufs=4, name="hb")
                nc.scalar.activation(out=hb[:, :], in_=hf[:, :], func=AF.Relu)
                hT.append(hb)
            for kt, (s, p) in enumerate(PT):
                pa = ps_p2.tile([128, OUT], FP, tag="pa", name="pa")
                for mh in range(2):
                    nc.tensor.matmul(
                        pa[:p, :], lhsT=hT[mh][:, s : s + p],
                        rhs=sp_sb[:128, SP_DW + mh * OUT : SP_DW + (mh + 1) * OUT],
                        start=(mh == 0), stop=(mh == 1),
                    )
                sa = scr.tile([128, OUT], FP, tag="p2sa", name="sa")
                nc.scalar.activation(
                    out=sa[:p, :], in_=pa[:p, :], func=AF.Copy, scale=fac[kt][:p, :]
                )
                pb = ps_p2.tile([128, OUT], FP, tag="pb", name="pb")
                nc.tensor.matmul(
                    pb[:p, :], lhsT=fT[:SIDE, s : s + p],
                    rhs=sp_sb[:SIDE, w2c : w2c + OUT],
                    start=True, stop=True,
                )
                so = scr.tile([128, OUT], FP, tag="p2so", name="so")
                nc.vector.tensor_tensor(
                    out=so[:p, :], in0=pb[:p, :], in1=sa[:p, :], op=ALU.add
                )
                ro = scr.tile([128, OUT], FP, tag="p2ro", name="ro")
                nc.scalar.activation(out=ro[:p, :], in_=so[:p, :], func=AF.Relu)
                nc.sync.dma_start(out=o_dram[s : s + p, :], in_=ro[:p, :])

        pass2(dram_hv_red, fT_v, SP_VW2, b_fac, BI, v_out, "v")
        pass2(dram_hu_red, fT_u, SP_UW2, a_fac, BU, u_out, "u")

    nc.compile()
    return nc


_CACHE = {}


def _get_program():
    if "nc" not in _CACHE:
        _CACHE["nc"] = build_program()
    return _CACHE["nc"]


def _pack_w(w_slice):
    # w_slice: [R, M, 500] bf16 -> [125, R*4*M] with col ((r*4+kt)*M + m)
    return np.ascontiguousarray(
        w_slice.reshape(R, M, 4, 125).transpose(3, 0, 2, 1).reshape(125, WCOLS)
    )


def make_in_maps(inputs):
    import ml_dtypes

    bf = ml_dtypes.bfloat16
    adj = np.asarray(inputs["adj_matrix"], dtype=np.int32)
    adjB = adj.astype(bf)  # values 0..5: exact in bf16
    msg_W = np.asarray(inputs["msg_W"], np.float32).astype(bf)
    u_sfT = np.asarray(inputs["u_sideFeat"], np.float32).astype(bf).T
    v_sfT = np.asarray(inputs["v_sideFeat"], np.float32).astype(bf).T
    ub1 = np.asarray(inputs["u_b1"], np.float32).reshape(SIDE, 1)
    vb1 = np.asarray(inputs["v_b1"], np.float32).reshape(SIDE, 1)

    sp = np.zeros((128, SP_COLS), bf)
    dw = np.asarray(inputs["dense_W"], np.float32).astype(bf)  # [75, 256]
    sp[:, SP_DW : SP_DW + 150] = dw.T.reshape(2, 128, OUT).transpose(1, 0, 2).reshape(
        128, 150
    )
    sp[:, SP_UW1 : SP_UW1 + SIDE] = np.asarray(inputs["u_W1"], np.float32).astype(bf).T
    sp[:, SP_VW1 : SP_VW1 + SIDE] = np.asarray(inputs["v_W1"], np.float32).astype(bf).T
    sp[:SIDE, SP_UW2 : SP_UW2 + OUT] = (
        np.asarray(inputs["u_W2"], np.float32).astype(bf).T
    )
    sp[:SIDE, SP_VW2 : SP_VW2 + OUT] = (
        np.asarray(inputs["v_W2"], np.float32).astype(bf).T
    )

    in_maps = []
    for c in range(NCORES):
        us, ie = c * BU, c * BI
        in_maps.append(
            {
                "adjR": np.ascontiguousarray(adjB[us : us + BU, :]),
                "adjCT": np.ascontiguousarray(adjB[:, ie : ie + BI].T),
                "wuH": _pack_w(msg_W[:, :, us : us + BU]),
                "wiH": _pack_w(msg_W[:, :, NU + ie : NU + ie + BI]),
                "ufT": np.ascontiguousarray(u_sfT[:, us : us + BU]),
                "vfT": np.ascontiguousarray(v_sfT[:, ie : ie + BI]),
                "smallpack": sp,
                "ub1": ub1,
                "vb1": vb1,
            }
        )
    return in_maps


def assemble(results):
    U = np.empty((NU, OUT), np.float32)
    V = np.empty((NI, OUT), np.float32)
    for c in range(NCORES):
        U[c * BU : (c + 1) * BU] = results[c]["u_out"]
        V[c * BI : (c + 1) * BI] = results[c]["v_out"]
    return (U, V)


def kernel(**inputs):
    from concourse.bass_utils import run_bass_kernel_spmd

    nc = _get_program()
    res = run_bass_kernel_spmd(nc, make_in_maps(inputs), core_ids=list(range(NCORES)))
    return assemble(res.results)


# revision 9
# speedup vs baseline: 1.6803x; 1.0470x over previous
"""Trainium2 Bass kernel for the bipartite GNN message-passing encoder.

Math (see reference.py):
  A_r = (adj == r), r = 1..5
  An_r = diag(1/sqrt(Nu)) A_r diag(1/sqrt(Nv))
  Hu = relu(sum_r An_r @ W_items_r^T)   [NU, M]
  Hv = relu(sum_r An_r^T @ W_users_r^T) [NI, M]
  U  = relu(Hu @ dense_W^T + relu(u_sideFeat @ u_W1^T + u_b1) @ u_W2^T)
  V  = relu(Hv @ dense_W^T + relu(v_sideFeat @ v_W1^T + v_b1) @ v_W2^T)

Sharding: symmetric 1D. Core c owns users U_c = [500c, 500c+500) and items
I_c = [500c, 500c+500). The host hands each core TWO adjacency views in
int8: adjR = adj[U_c, :] (full rows) and adjCT = adj[:, I_c]^T (full
columns, pre-transposed). Row degrees for U_c and column degrees for I_c
are therefore LOCAL - no degree collectives - so the pass-1 mask-matmul
streams start within ~8us of launch. Each stream produces a partial over
the full opposite side (HvT partial [M, NI] from my users; HuT partial
[M, NU] from my items), laid out in DRAM blocked by destination core
[8, M, 500] and combined with a single bf16 ReduceScatter each. Pass 2
is then fully local.

A 4-byte dummy AllReduce is triggered as the first instruction so the
collectives init barrier (which waits for the slowest core's trigger)
overlaps the local compute instead of delaying the first real
collective.

Engine budget: PE runs 640 back-to-back [<=125c x 128 x 500] bf16
matmuls (~165us at the 81% GPIO clock limit; no transposes - everything
arrives host-pre-transposed). DVE builds the 80 masks (dual-op is_equal
x per-partition degree factor) and the degree factors (fused
nz+rowsum via accum_out, then max+pow(-0.5)). ACT evacuates PSUM and
does pass-2 activations. DMA issue serializes per HW DGE queue at
~130GB/s, so the inputs ride TWO queues: sync gets adjR interleaved
with the per-kt weight chunks (first matmul operands land first);
scalar gets adjCT/wiH/side tensors (needed only from ~45us on).
"""

import sys

import numpy as np

if "/opt/trn_rl_repo" not in sys.path:
    sys.path.insert(0, "/opt/trn_rl_repo")

import concourse.bacc as bacc  # noqa: E402
import concourse.mybir as mybir  # noqa: E402
import concourse.tile as tile  # noqa: E402

FP = mybir.dt.float32
BF = mybir.dt.bfloat16
I8 = mybir.dt.int8

NU = NI = 4000
R = 5
M = 256
OUT = 75
SIDE = 64
FDIM = 128

NCORES = 8
BU = NU // NCORES  # 500 users per core
BI = NI // NCORES  # 500 items per core

AF = mybir.ActivationFunctionType
ALU = mybir.AluOpType

ALL_GROUP = [list(range(NCORES))]
PAIR_GROUPS = [[2 * a, 2 * a + 1] for a in range(NCORES // 2)]

PT = [(t * 125, 125) for t in range(4)]  # 4 partition tiles over 500
WK = R * M  # 1280 packed weight columns per kt chunk
# smallpack column layout
SP_DW = 0  # [128, 2x75] dense_W^T halves
SP_UW1 = 150  # [128, 64]
SP_VW1 = 214  # [128, 64]
SP_UW2 = 278  # [64, 75]
SP_VW2 = 353  # [64, 75]
SP_COLS = 428


def build_program():
    from contextlib import ExitStack

    nc = bacc.Bacc("TRN2", target_bir_lowering=False, debug=False, num_devices=NCORES)

    # ---- I/O ---- (all host-sliced / packed / pre-transposed)
    adjR = nc.dram_tensor("adjR", [BU, NI], I8, kind="ExternalInput")
    adjCT = nc.dram_tensor("adjCT", [BI, NU], I8, kind="ExternalInput")
    # packed msg_W: [4kt][125, R*M] with col (r*M + m)
    wuH = nc.dram_tensor("wuH", [4, 125, WK], BF, kind="ExternalInput")
    wiH = nc.dram_tensor("wiH", [4, 125, WK], BF, kind="ExternalInput")
    ufT = nc.dram_tensor("ufT", [FDIM, BU], BF, kind="ExternalInput")
    vfT = nc.dram_tensor("vfT", [FDIM, BI], BF, kind="ExternalInput")
    smallpack = nc.dram_tensor("smallpack", [128, SP_COLS], BF, kind="ExternalInput")
    ub1 = nc.dram_tensor("ub1", [SIDE, 1], FP, kind="ExternalInput")
    vb1 = nc.dram_tensor("vb1", [SIDE, 1], FP, kind="ExternalInput")
    u_out = nc.dram_tensor("u_out", [BU, OUT], FP, kind="ExternalOutput")
    v_out = nc.dram_tensor("v_out", [BI, OUT], FP, kind="ExternalOutput")

    with tile.TileContext(nc) as tc, ExitStack() as ctx:
        res = ctx.enter_context(tc.tile_pool(name="res", bufs=1))
        scr = ctx.enter_context(tc.tile_pool(name="scr", bufs=2))
        dram = ctx.enter_context(tc.tile_pool(name="dram", bufs=1, space="DRAM"))

        # ---- dummy collective: absorbs the init barrier during compute ----
        dummy_src = res.tile([1, 8], FP, tag="dummy_src")
        nc.gpsimd.memset(dummy_src[:], 0.0)
        dram_dmy = dram.tile([1, 8], FP, tag="dram_dmy")
        dram_dmy_o = dram.tile([1, 8], FP, tag="dram_dmy_o")
        nc.scalar.dma_start(out=dram_dmy[:, :], in_=dummy_src[:, :])
        nc.gpsimd.collective_compute(
            "AllReduce", ALU.add, replica_groups=PAIR_GROUPS,
            ins=[dram_dmy.opt()], outs=[dram_dmy_o.opt()],
        )

        # ---- input DMAs: sync = adjR + wu chunks interleaved (kt order) ----
        adjR_t, wu_sb = [], []
        for kt, (s, p) in enumerate(PT):
            ab = res.tile([128, NI], I8, tag=f"aR{kt}", name="ab")
            nc.sync.dma_start(out=ab[:p, :], in_=adjR[s : s + p, :])
            adjR_t.append(ab)
            wt = res.tile([125, WK], BF, tag=f"wu{kt}", name="wt")
            nc.sync.dma_start(out=wt[:, :], in_=wuH[kt, :, :])
            wu_sb.append(wt)

        # ---- scalar-queue tensors: tiles allocated now, DMAs emitted inside
        # pass-1 preps (interleaved with the degree sqrts so neither blocks)
        adjCT_t = [
            res.tile([128, NU], I8, tag=f"aC{kt}", name="ac") for kt in range(4)
        ]
        wi_sb = [res.tile([125, WK], BF, tag=f"wi{kt}", name="wt") for kt in range(4)]
        ufT_sb = res.tile([128, BU], BF, tag="ufT_sb")
        vfT_sb = res.tile([128, BI], BF, tag="vfT_sb")
        sp_sb = res.tile([128, SP_COLS], BF, tag="sp_sb")
        ub1_t = res.tile([SIDE, 1], FP, tag="ub1_t")
        vb1_t = res.tile([SIDE, 1], FP, tag="vb1_t")

        def wsl(w_sb, r, kt, mh):  # packed lhsT slice [125, 128]
            c = r * M + mh * 128
            return w_sb[kt][:125, c : c + 128]

        # ---- local degree factors (sqrt on ACT), emitted lazily ----
        a_fac = [None] * 4
        b_fac = [None] * 4

        def emit_deg(adj_t, fac, kt, nm):
            p = PT[kt][1]
            nz = scr.tile([128, NI], BF, tag="nz", bufs=2, name="nz")
            dg = scr.tile([128, 1], FP, tag="dg", bufs=2, name="dg")
            nc.vector.tensor_scalar(
                out=nz[:p, :], in0=adj_t[kt][:p, :], scalar1=1.0, scalar2=0.0,
                op0=ALU.min, op1=ALU.add, accum_out=dg[:p, :],
            )
            m1 = scr.tile([128, 1], FP, tag="m1", bufs=2, name="m1")
            nc.vector.tensor_scalar(
                out=m1[:p, :], in0=dg[:p, :], scalar1=1.0, scalar2=None, op0=ALU.max,
            )
            sq = scr.tile([128, 1], FP, tag="sq", bufs=2, name="sq")
            nc.scalar.sqrt(out=sq[:p, :], in_=m1[:p, :])
            fc = res.tile([128, 1], FP, tag=f"{nm}fac{kt}", name="fc")
            nc.vector.reciprocal(out=fc[:p, :], in_=sq[:p, :])
            fac[kt] = fc

        ps_mm = tc.alloc_tile_pool(name="ps_mm", bufs=1, space="PSUM")

        # DRAM partial buffers, blocked by destination core [8, M, 500]
        dram_hv = dram.tile([NCORES, M, BI], BF, tag="dram_hv")
        dram_hu = dram.tile([NCORES, M, BU], BF, tag="dram_hu")
        dram_hv_red = dram.tile([M, BI], BF, tag="dram_hv_red")
        dram_hu_red = dram.tile([M, BU], BF, tag="dram_hu_red")

        # ---- pass 1: one side = 2 halves x (4kt x 5r masks -> 8-bank matmul) ----
        def pass1(adj_t, fac, w_sb, w_blk, dram_part, prep):
            # partial H^T[m, col] = sum_r sum_p (fac_p * mask_r[p, col]) * W[r][m, p]
            for h in range(2):
                P = [
                    [
                        ps_mm.tile([128, w_blk], FP, tag=f"p{mh}{cc}", name="P")
                        for cc in range(4)
                    ]
                    for mh in range(2)
                ]
                for kt, (s, p) in enumerate(PT):
                    if prep is not None:
                        prep(h, kt)
                    for r in range(R):
                        msk = scr.tile(
                            [128, 4 * w_blk], BF, tag="mask", bufs=3, name="msk"
                        )
                        nc.vector.tensor_scalar(
                            out=msk[:p, :],
                            in0=adj_t[kt][:p, h * 4 * w_blk : (h + 1) * 4 * w_blk],
                            scalar1=float(r + 1), scalar2=fac[kt][:p, :],
                            op0=ALU.is_equal, op1=ALU.mult,
                        )
                        first = kt == 0 and r == 0
                        last = kt == 3 and r == R - 1
                        for mh in range(2):
                            for cc in range(4):
                                nc.tensor.matmul(
                                    P[mh][cc][:, :],
                                    lhsT=wsl(w_sb, r, kt, mh),
                                    rhs=msk[:p, cc * w_blk : (cc + 1) * w_blk],
                                    start=first, stop=last,
                                )
                # evacuate in matmul emission order so the next half's first
                # matmul only waits on its own bank
                for mh in range(2):
                    for cc in range(4):
                        ev = scr.tile([128, w_blk], BF, tag="ev", bufs=4, name="ev")
                        nc.scalar.copy(out=ev[:, :], in_=P[mh][cc][:, :])
                        nc.sync.dma_start(
                            out=dram_part[h * 4 + cc, mh * 128 : (mh + 1) * 128, :],
                            in_=ev[:, :],
                        )

        def item_prep(h, kt):
            if h == 0:
                # aR degree chain, then this kt's adjCT load right behind the
                # sqrt in the scalar FIFO (executes while DVE builds masks)
                emit_deg(adjR_t, a_fac, kt, "a")
                s, p = PT[kt]
                nc.scalar.dma_start(
                    out=adjCT_t[kt][:p, :], in_=adjCT[s : s + p, :]
                )
            else:
                emit_deg(adjCT_t, b_fac, kt, "b")
                if kt == 0:
                    for k2 in range(4):
                        nc.scalar.dma_start(out=wi_sb[k2][:, :], in_=wiH[k2, :, :])
                elif kt == 1:
                    nc.scalar.dma_start(out=ufT_sb[:, :], in_=ufT[:, :])
                    nc.scalar.dma_start(out=vfT_sb[:, :], in_=vfT[:, :])
                elif kt == 2:
                    nc.scalar.dma_start(out=sp_sb[:, :], in_=smallpack[:, :])
                    nc.scalar.dma_start(out=ub1_t[:, :], in_=ub1[:, :])
                    nc.scalar.dma_start(out=vb1_t[:, :], in_=vb1[:, :])

        pass1(adjR_t, a_fac, wu_sb, BI, dram_hv, item_prep)
        nc.gpsimd.collective_compute(
            "ReduceScatter", ALU.add, replica_groups=ALL_GROUP,
            ins=[dram_hv.opt()], outs=[dram_hv_red.opt()],
        )
        pass1(adjCT_t, b_fac, wi_sb, BU, dram_hu, None)
        nc.gpsimd.collective_compute(
            "ReduceScatter", ALU.add, replica_groups=ALL_GROUP,
            ins=[dram_hu.opt()], outs=[dram_hu_red.opt()],
        )

        ps_mm.release()
        ps_p2 = ctx.enter_context(tc.tile_pool(name="ps_p2", bufs=2, space="PSUM"))

        # ---- side-feature heads (PE is free once pass 1 drains) ----
        def side_head(w1c, bia, sft, n, nm):
            fT = res.tile([SIDE, n], BF, tag=f"fT_{nm}", name="fT")
            pf = ps_p2.tile([SIDE, n], FP, tag="pf", name="pf")
            nc.tensor.matmul(
                pf[:, :], lhsT=sp_sb[:FDIM, w1c : w1c + SIDE], rhs=sft[:FDIM, :],
                start=True, stop=True,
            )
            nc.scalar.activation(
                out=fT[:, :], in_=pf[:, :], func=AF.Relu, bias=bia[:, :],
            )
            return fT

        fT_v = side_head(SP_VW1, vb1_t, vfT_sb, BI, "v")
        fT_u = side_head(SP_UW1, ub1_t, ufT_sb, BU, "u")

        # ---- pass 2 (fully local): out = relu(fac*relu(H)@dW^T + F@W2^T) ----
        def pass2(h_red, fT, w2c, fac, n, o_dram, nm):
            hT = []
            for mh in range(2):
                hf = scr.tile([128, n], BF, tag="p2h", bufs=4, name="hf")
                nc.sync.dma_start(
                    out=hf[:, :], in_=h_red[mh * 128 : (mh + 1) * 128, :]
                )
                hb = scr.tile([128, n], BF, tag="p2hb", bufs=4, name="hb")
                nc.scalar.activation(out=hb[:, :], in_=hf[:, :], func=AF.Relu)
                hT.append(hb)
            for kt, (s, p) in enumerate(PT):
                pa = ps_p2.tile([128, OUT], FP, tag="pa", name="pa")
                for mh in range(2):
                    nc.tensor.matmul(
                        pa[:p, :], lhsT=hT[mh][:, s : s + p],
                        rhs=sp_sb[:128, SP_DW + mh * OUT : SP_DW + (mh + 1) * OUT],
                        start=(mh == 0), stop=(mh == 1),
                    )
                sa = scr.tile([128, OUT], FP, tag="p2sa", name="sa")
                nc.scalar.activation(
                    out=sa[:p, :], in_=pa[:p, :], func=AF.Copy, scale=fac[kt][:p, :]
                )
                pb = ps_p2.tile([128, OUT], FP, tag="pb", name="pb")
                nc.tensor.matmul(
                    pb[:p, :], lhsT=fT[:SIDE, s : s + p],
                    rhs=sp_sb[:SIDE, w2c : w2c + OUT],
                    start=True, stop=True,
                )
                so = scr.tile([128, OUT], FP, tag="p2so", name="so")
                nc.vector.tensor_tensor(
                    out=so[:p, :], in0=pb[:p, :], in1=sa[:p, :], op=ALU.add
                )
                ro = scr.tile([128, OUT], FP, tag="p2ro", name="ro")
                nc.scalar.activation(out=ro[:p, :], in_=so[:p, :], func=AF.Relu)
                nc.sync.dma_start(out=o_dram[s : s + p, :], in_=ro[:p, :])

        pass2(dram_hv_red, fT_v, SP_VW2, b_fac, BI, v_out, "v")
        pass2(dram_hu_red, fT_u, SP_UW2, a_fac, BU, u_out, "u")

    nc.compile()
    return nc


_CACHE = {}


def _get_program():
    if "nc" not in _CACHE:
        _CACHE["nc"] = build_program()
    return _CACHE["nc"]


def _pack_w(w_slice):
    # w_slice: [R, M, 500] bf16 -> [4, 125, R*M] with chunk kt, col (r*M + m)
    return np.ascontiguousarray(
        w_slice.reshape(R, M, 4, 125).transpose(2, 3, 0, 1).reshape(4, 125, R * M)
    )


def make_in_maps(inputs):
    import ml_dtypes

    bf = ml_dtypes.bfloat16
    adj = np.asarray(inputs["adj_matrix"], dtype=np.int32)
    adjB = adj.astype(np.int8)  # values 0..5
    msg_W = np.asarray(inputs["msg_W"], np.float32).astype(bf)
    u_sfT = np.asarray(inputs["u_sideFeat"], np.float32).astype(bf).T
    v_sfT = np.asarray(inputs["v_sideFeat"], np.float32).astype(bf).T
    ub1 = np.asarray(inputs["u_b1"], np.float32).reshape(SIDE, 1)
    vb1 = np.asarray(inputs["v_b1"], np.float32).reshape(SIDE, 1)

    sp = np.zeros((128, SP_COLS), bf)
    dw = np.asarray(inputs["dense_W"], np.float32).astype(bf)  # [75, 256]
    sp[:, SP_DW : SP_DW + 150] = dw.T.reshape(2, 128, OUT).transpose(1, 0, 2).reshape(
        128, 150
    )
    sp[:, SP_UW1 : SP_UW1 + SIDE] = np.asarray(inputs["u_W1"], np.float32).astype(bf).T
    sp[:, SP_VW1 : SP_VW1 + SIDE] = np.asarray(inputs["v_W1"], np.float32).astype(bf).T
    sp[:SIDE, SP_UW2 : SP_UW2 + OUT] = (
        np.asarray(inputs["u_W2"], np.float32).astype(bf).T
    )
    sp[:SIDE, SP_VW2 : SP_VW2 + OUT] = (
        np.asarray(inputs["v_W2"], np.float32).astype(bf).T
    )

    in_maps = []
    for c in range(NCORES):
        us, ie = c * BU, c * BI
        in_maps.append(
            {
                "adjR": np.ascontiguousarray(adjB[us : us + BU, :]),
                "adjCT": np.ascontiguousarray(adjB[:, ie : ie + BI].T),
                "wuH": _pack_w(msg_W[:, :, us : us + BU]),
                "wiH": _pack_w(msg_W[:, :, NU + ie : NU + ie + BI]),
                "ufT": np.ascontiguousarray(u_sfT[:, us : us + BU]),
                "vfT": np.ascontiguousarray(v_sfT[:, ie : ie + BI]),
                "smallpack": sp,
                "ub1": ub1,
                "vb1": vb1,
            }
        )
    return in_maps


def assemble(results):
    U = np.empty((NU, OUT), np.float32)
    V = np.empty((NI, OUT), np.float32)
    for c in range(NCORES):
        U[c * BU : (c + 1) * BU] = results[c]["u_out"]
        V[c * BI : (c + 1) * BI] = results[c]["v_out"]
    return (U, V)


def kernel(**inputs):
    from concourse.bass_utils import run_bass_kernel_spmd

    nc = _get_program()
    res = run_bass_kernel_spmd(nc, make_in_maps(inputs), core_ids=list(range(NCORES)))
    return assemble(res.results)
